# revision 1
# baseline (speedup 1.0000x reference)
"""CIN (Compressed Interaction Network) kernel for Trainium2, SPMD over 8 cores.

Reference computation (per layer l, with x0 = embeddings (B, M, D)):
    xk = relu(einsum("bmd,bhd,mhk->bkd", x0, x_{k-1}, W_l))   # (B, K, D)
    out_l = sum_d xk                                           # (B, K)
Output: concat(out_0, out_1, out_2) -> (B, 192)

Mapping (per core, B_loc = 2048 batch rows, data-parallel over B):
  * Interaction index (m,h) / output index k live on SBUF partitions;
    (b, d) is flattened on the free dim (N = BT*D per b-tile).
  * For each 128-row block g of the (m,h) interaction space:
      bc_g   = x0[m(p)] broadcast per partition  (DMA from DRAM with a
               zero-stride access pattern -- no compute engine involved)
      v_g    = bc_g * XkRep       (bf16 tensor_tensor at 2x, DVE/GPSIMD)
      out   += Wdup_g.T @ v_g     (PE, fp32 PSUM accumulation)
  * Wdup_g = [W_g | W_g] (host-duplicated) so the PSUM result lands
    duplicated in both partition halves -> after ReLU the SBUF tile is
    directly the next layer's replicated Xk (XkRep[p] = xk[p % 64]).
  * ReLU via ScalarE PSUM->SBUF (bf16); per-layer d-sums via DVE reduce;
    outputs stored k-major (192, B_loc), transposed/concatenated on host.

All matmul/TT data is bf16 (fp32 PSUM accumulation); norm rel err ~2e-3.

Self-contained: hardcodes shapes from the problem spec.
"""

import os

import ml_dtypes
import numpy as np

B, M, D = 16384, 32, 16
N_CORES = 8
B_LOC = B // N_CORES

BT = 64  # batch rows per b-tile
N_FREE = BT * D  # 1024 free elems per b-tile
N_TILES = B_LOC // BT
MM_FREE = 512  # one fp32 PSUM bank; max free dim per matmul

N_TILES_BUILD = int(os.environ.get("CIN_NTILES", str(N_TILES)))
DEV_REPS = int(os.environ.get("CIN_DEVREPS", "1"))  # on-device repeat loop (timing only)
ABL_NO_TT = int(os.environ.get("CIN_ABL_NO_TT", "0"))    # matmul reads bc directly
ABL_NO_MM = int(os.environ.get("CIN_ABL_NO_MM", "0"))    # skip matmuls+relu+reduce
ABL_NO_BC = int(os.environ.get("CIN_ABL_NO_BC", "0"))    # TT reads xt instead of bc (skip bc DMAs)
# every GPSIMD_EVERY-th interaction block's multiply goes to GPSIMD
GPSIMD_EVERY = int(os.environ.get("CIN_GPSIMD_EVERY", "4"))
GPSIMD_TAIL = int(os.environ.get("CIN_GPSIMD_TAIL", "0"))  # 0: strided; n>0: last n blocks per layer
VBUFS = int(os.environ.get("CIN_VBUFS", "12"))
RED_DELAY = int(os.environ.get("CIN_RED_DELAY", "400"))
BC_EARLY = int(os.environ.get("CIN_BC_EARLY", "200"))
OUTPS_BUFS = int(os.environ.get("CIN_OUTPS", "2"))
BC64_BUFS = int(os.environ.get("CIN_BC64BUFS", "14"))
BC32_BUFS = int(os.environ.get("CIN_BC32BUFS", "5"))

_CACHE = {}


def _prep_weights(W0, W1, W2):
    """Flatten (m,h)->rows, split into 128-row blocks, duplicate along k."""
    out = {}
    for i, W in enumerate((W0, W1, W2)):
        m, h, k = W.shape
        flat = np.ascontiguousarray(np.asarray(W, dtype=np.float32).reshape(m * h, k))
        G = (m * h) // 128
        blocks = flat.reshape(G, 128, k)
        dup = np.concatenate([blocks, blocks], axis=2)  # (G, 128, 128)
        out[f"w{i}dup"] = np.ascontiguousarray(dup.astype(ml_dtypes.bfloat16))
    return out


def _build_bass():
    import concourse.bass as bass
    import concourse.mybir as mybir
    import concourse.tile as tile
    from concourse import bacc

    f32 = mybir.dt.float32
    bf16 = mybir.dt.bfloat16

    nc = bacc.Bacc(None, target_bir_lowering=False, debug=False)

    # x0 transposed to (m, b*d), bf16
    x0t = nc.dram_tensor("x0t", (M, B_LOC * D), bf16, kind="ExternalInput")
    w_dram = [
        nc.dram_tensor("w0dup", (8, 128, 128), bf16, kind="ExternalInput"),
        nc.dram_tensor("w1dup", (16, 128, 128), bf16, kind="ExternalInput"),
        nc.dram_tensor("w2dup", (16, 128, 128), bf16, kind="ExternalInput"),
    ]
    out_dram = nc.dram_tensor("out", (192, B_LOC), f32, kind="ExternalOutput")

    ROW = B_LOC * D  # x0t row stride in elements

    with tile.TileContext(nc) as tc:
        with (
            tc.tile_pool(name="consts", bufs=1) as consts,
            tc.tile_pool(name="xin", bufs=2) as xin,
            tc.tile_pool(name="bc64p", bufs=BC64_BUFS) as bc64p,
            tc.tile_pool(name="bc32p", bufs=BC32_BUFS) as bc32p,
            tc.tile_pool(name="xk", bufs=6) as xkp,
            tc.tile_pool(name="vbuf", bufs=VBUFS) as vbuf,
            tc.tile_pool(name="obuf", bufs=4) as obuf,
            tc.tile_pool(name="outps", bufs=OUTPS_BUFS, space="PSUM") as outps,
        ):
            w_sb = []
            for i, (wd, G) in enumerate(zip(w_dram, (8, 16, 16))):
                t = consts.tile([128, G, 128], bf16, tag=f"w{i}")
                nc.sync.dma_start(out=t, in_=wd.rearrange("g p q -> p g q"))
                w_sb.append(t)

            def load_tile(t_i):
                off = t_i * N_FREE
                import contextlib
                bccm = (lambda: tc.high_priority(offset=BC_EARLY)) if BC_EARLY else contextlib.nullcontext
                xt = xin.tile([128, N_FREE], bf16, tag="x0x4")
                for s in range(4):
                    with bccm():
                        nc.sync.dma_start(
                            out=xt[32 * s : 32 * (s + 1), :],
                            in_=x0t[:, off : off + N_FREE],
                        )
                bc32, bc64 = [], []
                for q in range(2):  # bc32 quads: blocks 4q..4q+3
                    t = bc32p.tile([128, 4, N_FREE], bf16, tag="bc32")
                    for a in range(4):
                        with bccm():
                            nc.sync.dma_start(
                                out=t[32 * a : 32 * (a + 1), :, :],
                                in_=bass.AP(
                                    tensor=x0t,
                                    offset=(16 * q + a) * ROW + off,
                                    ap=[[0, 32], [4 * ROW, 4], [1, N_FREE]],
                                ),
                            )
                    bc32.extend(t[:, j, :] for j in range(4))
                for q in range(4):  # bc64 quads: blocks 4q..4q+3
                    t = bc64p.tile([128, 4, N_FREE], bf16, tag="bc64")
                    for a in range(2):
                        with bccm():
                            nc.sync.dma_start(
                                out=t[64 * a : 64 * (a + 1), :, :],
                                in_=bass.AP(
                                    tensor=x0t,
                                    offset=(8 * q + a) * ROW + off,
                                    ap=[[0, 64], [2 * ROW, 4], [1, N_FREE]],
                                ),
                            )
                    bc64.extend(t[:, j, :] for j in range(4))
                outs_all = obuf.tile([64, 3, BT], f32, tag="outs")
                return {"xt": xt, "bc32": bc32, "bc64": bc64, "t_i": t_i,
                        "xk_rep": xt, "pending": None, "outs": outs_all,
                        "n_red": 0}

            def emit_reduce(st, layer, xk_tile):
                import contextlib
                cm = tc.high_priority(offset=-RED_DELAY) if RED_DELAY else contextlib.nullcontext()
                with cm:
                    nc.vector.reduce_sum(
                        out=st["outs"][:, layer, :],
                        in_=xk_tile[:64].rearrange("k (b d) -> k b d", d=D),
                        axis=mybir.AxisListType.X,
                    )
                st["n_red"] += 1
                if st["n_red"] == 3:
                    nc.sync.dma_start(
                        out=bass.AP(
                            tensor=out_dram,
                            offset=st["t_i"] * BT,
                            ap=[[B_LOC, 64], [64 * B_LOC, 3], [1, BT]],
                        ),
                        in_=st["outs"],
                    )

            def emit_layer(st, layer):
                G = 8 if layer == 0 else 16
                W = w_sb[layer]
                bcs = st["bc32"] if layer == 0 else st["bc64"]
                xk_rep = st["xk_rep"]
                ops = outps.tile([128, N_FREE], f32, tag="outps")
                for g in range(G):
                    src_in0 = st["xt"] if ABL_NO_BC else bcs[g]
                    if ABL_NO_TT:
                        v = src_in0
                    else:
                        v = vbuf.tile([128, N_FREE], bf16, tag="v")
                        if GPSIMD_TAIL > 0:
                            on_gp = g >= G - GPSIMD_TAIL
                        else:
                            on_gp = g % GPSIMD_EVERY == GPSIMD_EVERY - 1
                        if on_gp:
                            nc.gpsimd.tensor_mul(v, src_in0, xk_rep)
                        else:
                            nc.vector.tensor_mul(v, src_in0, xk_rep)
                    if not ABL_NO_MM:
                        for h0 in range(0, N_FREE, MM_FREE):
                            nc.tensor.matmul(
                                ops[:, h0 : h0 + MM_FREE],
                                lhsT=W[:, g, :],
                                rhs=v[:, h0 : h0 + MM_FREE],
                                start=(g == 0),
                                stop=(g == G - 1),
                            )
                    if st["pending"] is not None and g == G // 2:
                        emit_reduce(st, *st["pending"])
                        st["pending"] = None
                if ABL_NO_MM:
                    st["xk_rep"] = st["xt"]
                    if st["pending"] is None:
                        st["pending"] = (layer, st["xt"])
                    return
                xk_new = xkp.tile([128, N_FREE], bf16, tag="xk")
                nc.scalar.activation(xk_new, ops, mybir.ActivationFunctionType.Relu)
                st["pending"] = (layer, xk_new)
                st["xk_rep"] = xk_new

            def whole_pass():
                for pair in range(N_TILES_BUILD // 2):
                    stA = load_tile(2 * pair)
                    stB = load_tile(2 * pair + 1)
                    for layer in range(3):
                        emit_layer(stA, layer)
                        emit_layer(stB, layer)
                    emit_reduce(stA, *stA["pending"])
                    emit_reduce(stB, *stB["pending"])

            if DEV_REPS > 1:
                with tc.For_i(0, DEV_REPS, 1):
                    whole_pass()
            else:
                whole_pass()

    nc.finalize()
    return nc


def _get_program():
    if "nc" not in _CACHE:
        _CACHE["nc"] = _build_bass()
    return _CACHE["nc"]


def kernel(embeddings, W0, W1, W2):
    from concourse.bass_utils import run_bass_kernel_spmd

    embeddings = np.asarray(embeddings, dtype=np.float32)
    wmaps = _prep_weights(np.asarray(W0), np.asarray(W1), np.asarray(W2))

    in_maps = []
    for c in range(N_CORES):
        emb = embeddings[c * B_LOC : (c + 1) * B_LOC]  # (B_LOC, M, D)
        x0t = np.ascontiguousarray(
            emb.transpose(1, 0, 2).reshape(M, B_LOC * D).astype(ml_dtypes.bfloat16)
        )
        in_maps.append({"x0t": x0t, **wmaps})

    nc = _get_program()
    res = run_bass_kernel_spmd(nc, in_maps, core_ids=list(range(N_CORES)))
    if res.exec_time_ns is not None:
        _CACHE["exec_time_ns"] = res.exec_time_ns

    outs = [r["out"].T for r in res.results]  # each (B_LOC, 192)
    return np.ascontiguousarray(np.concatenate(outs, axis=0))

